# revision 59
# baseline (speedup 1.0000x reference)
"""BiLSTM-CRF NLL kernel for 8 trn2 NeuronCores (data-parallel over batch).

Per core (B_loc=16 sequences):
  token index  = s*16 + b                      (s-major, b-minor)
  Embeddings are gathered + transposed to xT [128(E), token] bf16; the input
  projection runs per recurrence step as 8 small PE matmuls straight into the
  gate PSUM tile (PE is otherwise idle), with the bias injected by a
  broadcast-matrix matmul.  All gate activations are Tanh (same act-table set
  as Exp, so emissions interleave without table reloads): sigmoid gates use
  tanh(a/2) with the 1/2 folded into host-prepped weights; the kernel carries
  cs = 2c and h'' = 2h, compensated in w_hh (x0.5) and w_em (x0.5).
  Per step: 17 PE matmuls, 3 ACT tanh (f/i/g on the chain, o hidden in the
  DVE window, then tanh(c)), 4 DVE ops; ~1.27us/step chain.
  h_all        [128(H), (d, s, b)]  bf16       both directions stored s-ordered
  CRF          exp-space chunked scan in bf16: 8 chunks x 32 steps as two
               independent 64-column chains, state [81(i*9+j), (c*16+b)],
               E81 block-diag and the ee81 broadcast built via selector
               matmuls (no DMA broadcasts).
"""

import math
import numpy as np
from contextlib import ExitStack

V, E, H, T = 30000, 128, 128, 9
B, S = 128, 256
NCORES = 8
BL = B // NCORES            # 16 sequences/core
NTOK = BL * S               # 4096 tokens/core
GORD = [1, 0, 2, 3]         # (f,i,g,o) expressed in torch gate order (i,f,g,o)
K0LOG = math.log(9.0)
NCH = 8                     # time chunks
CL = S // NCH               # 32 steps/chunk
CTOK = NTOK // NCH          # 512 tokens/chunk

_NC_CACHE = {}


def _build_program():
    import concourse.bass as bass
    import concourse.tile as tile
    from concourse import bacc, mybir

    f32 = mybir.dt.float32
    bf16 = mybir.dt.bfloat16
    i32 = mybir.dt.int32
    i16 = mybir.dt.int16
    AF = mybir.ActivationFunctionType
    ALU = mybir.AluOpType
    AP = bass.AP

    nc = bacc.Bacc("TRN2", target_bir_lowering=False, debug=False,
                   num_devices=NCORES)

    d_ids = nc.dram_tensor("ids16", [BL, S], i16, kind="ExternalInput").ap()
    d_tags = nc.dram_tensor("tags", [BL, S], i32, kind="ExternalInput").ap()
    d_embed = nc.dram_tensor("embed", [V, E], f32, kind="ExternalInput").ap()
    d_wihT = nc.dram_tensor("wihT", [E, 8 * H], f32, kind="ExternalInput").ap()
    d_whhT = nc.dram_tensor("whhT", [H, 8 * H], f32, kind="ExternalInput").ap()
    d_biasf = nc.dram_tensor("biasf", [H, 8], f32, kind="ExternalInput").ap()
    d_wemT = nc.dram_tensor("wemT", [2 * H, T], f32, kind="ExternalInput").ap()
    d_bem = nc.dram_tensor("bem", [T, 1], f32, kind="ExternalInput").ap()
    d_sten = nc.dram_tensor("sten", [2, T], f32, kind="ExternalInput").ap()
    d_trans = nc.dram_tensor("trans", [T, T], f32, kind="ExternalInput").ap()
    # structural constants (input-independent): maskA [81,81] + SEL16 [128,16]
    d_cf = nc.dram_tensor("cf32", [128, 97], f32, kind="ExternalInput").ap()
    # B9sel [9,81] + I81 [81,81] + g0diag [81,1]  (as f32, converted on-chip)
    d_cb = nc.dram_tensor("cbf32", [128, 163], f32, kind="ExternalInput").ap()
    d_out = nc.dram_tensor("out", [1, 1], f32, kind="ExternalOutput").ap()

    P = 128

    with tile.TileContext(nc) as tc, ExitStack() as ctx:
        consts = ctx.enter_context(tc.tile_pool(name="consts", bufs=1))
        big = ctx.enter_context(tc.tile_pool(name="big", bufs=1))
        stage = ctx.enter_context(tc.tile_pool(name="stage", bufs=2))
        rec = ctx.enter_context(tc.tile_pool(name="rec", bufs=3))
        scratch = ctx.enter_context(tc.tile_pool(name="scratch", bufs=1))

        # ================= constants =================
        ic = consts.tile([P, P], i32)
        ip = consts.tile([P, P], i32)
        nc.gpsimd.iota(ic[:], [[1, P]], base=0, channel_multiplier=0)
        nc.gpsimd.iota(ip[:], [[0, P]], base=0, channel_multiplier=1)
        I128f = consts.tile([P, P], f32)
        I128b = consts.tile([P, P], bf16)
        nc.vector.tensor_tensor(I128f[:], ic[:], ip[:], ALU.is_equal)
        nc.vector.tensor_tensor(I128b[:], ic[:], ip[:], ALU.is_equal)
        I9f = consts.tile([T, T], f32)
        nc.vector.tensor_tensor(I9f[:], ic[0:T, 0:T], ip[0:T, 0:T], ALU.is_equal)
        I81f = consts.tile([81, 81], f32)
        nc.vector.tensor_tensor(I81f[:], ic[0:81, 0:81], ip[0:81, 0:81],
                                ALU.is_equal)
        iota9 = consts.tile([P, T], i32)
        nc.gpsimd.iota(iota9[:], [[1, T]], base=0, channel_multiplier=0)
        ones1 = consts.tile([1, P], f32)
        nc.vector.memset(ones1[:], 1.0)

        ids_sb = consts.tile([P, S], i16)
        nc.vector.memset(ids_sb[:], 0)
        nc.sync.dma_start(ids_sb[0:BL, :], d_ids)
        tags_sb = consts.tile([BL, S], i32)
        nc.sync.dma_start(tags_sb[:], d_tags)
        CF = consts.tile([P, 97], f32)
        nc.sync.dma_start(CF[:], d_cf)
        maskA = CF[:, 0:81]                  # rows 0:81 meaningful
        SEL16 = CF[:, 81:97]
        CBs = stage.tile([P, 163], f32, tag="cbs", name="CBs")
        nc.sync.dma_start(CBs[:], d_cb)
        CB = consts.tile([P, 163], bf16)
        nc.vector.tensor_copy(CB[:], CBs[:])
        B9sel = CB[:, 0:81]                  # rows 0:9 meaningful
        I81b = CB[:, 81:162]                 # rows 0:81 meaningful
        g0diag = CB[:, 162:163]              # rows 0:81 meaningful
        B9f = consts.tile([T, 81], f32)
        nc.vector.tensor_copy(B9f[:], CBs[0:T, 0:81])

        wstage = stage.tile([P, 8 * H], f32, tag="wstage", name="wstage")
        nc.sync.dma_start(wstage[:], d_wihT)
        wih = consts.tile([P, 8 * H], bf16)
        nc.vector.tensor_copy(wih[:], wstage[:])
        wstage2 = stage.tile([P, 8 * H], f32, tag="wstage", name="wstage2")
        nc.sync.dma_start(wstage2[:], d_whhT)
        whh = consts.tile([P, 8 * H], bf16)
        nc.vector.tensor_copy(whh[:], wstage2[:])

        biasf = consts.tile([P, 8], f32)
        nc.sync.dma_start(biasf[:], d_biasf)
        bias128 = consts.tile([P, 8 * BL], bf16)
        nc.vector.tensor_copy(
            bias128.rearrange("p (g b) -> p g b", b=BL),
            biasf.unsqueeze(2).broadcast_to([P, 8, BL]))

        wemstage = stage.tile([P, 2 * T], f32, tag="wemstage", name="wemstage")
        nc.sync.dma_start(wemstage[:, 0:T], d_wemT[0:H, :])
        nc.sync.dma_start(wemstage[:, T:2 * T], d_wemT[H:2 * H, :])
        wem = consts.tile([P, 2 * T], bf16)
        nc.vector.tensor_copy(wem[:], wemstage[:])

        bem_sb = consts.tile([T, 1], f32)
        nc.sync.dma_start(bem_sb[:], d_bem)
        st_sb = consts.tile([1, T], f32)
        nc.sync.dma_start(st_sb[:], d_sten[0:1, :])
        en_sb = consts.tile([1, T], f32)
        nc.sync.dma_start(en_sb[:], d_sten[1:2, :])
        tr9 = consts.tile([T, T], f32)
        nc.sync.dma_start(tr9[:], d_trans)
        tr9T = consts.tile([T, T], f32)
        nc.sync.dma_start(tr9T[:], AP(d_trans.tensor, 0, [[1, T], [T, T]]))


        # ---- tag-derived prep (early; SP/Pool/DMA engines idle here) ----
        TB = consts.tile([P, 32], i32)                   # tags in token layout
        for sl in range(8):
            nc.sync.dma_start(
                TB[sl * BL:(sl + 1) * BL, :],
                AP(tags_sb.tensor, tags_sb.offset + sl,
                   [[tags_sb.ap[0][0], BL], [8, 32]]))
        TBn = consts.tile([P, 32], i32)                  # next tags, pad -1
        nc.vector.memset(TBn[:], -1)
        for sl in range(8):
            ncols = 31 if sl == 7 else 32
            nc.sync.dma_start(
                TBn[sl * BL:(sl + 1) * BL, 0:ncols],
                AP(tags_sb.tensor, tags_sb.offset + sl + 1,
                   [[tags_sb.ap[0][0], BL], [8, ncols]]))
        pi32 = consts.tile([P, 32], i32)                 # 9*tag_s + tag_{s+1}
        nc.vector.scalar_tensor_tensor(pi32[:], TB[:], 9, TBn[:],
                                       ALU.mult, ALU.add)
        iota81 = consts.tile([P, 81], i32)
        nc.gpsimd.iota(iota81[:], [[1, 81]], base=0, channel_multiplier=0)
        trrow = consts.tile([1, 81], f32)
        nc.sync.dma_start(trrow[:], AP(d_trans.tensor, 0, [[81, 1], [1, 81]]))

        # ---- E81 (block-diag exp(trans - ln9)) via selector matmuls ----
        tr9bT = consts.tile([T, T], bf16)
        nc.vector.tensor_copy(tr9bT[:], tr9T[:])
        negln9 = consts.tile([81, 1], f32)
        nc.vector.memset(negln9[:], -K0LOG)
        E81b = consts.tile([81, 81], bf16)
        with tc.tile_pool(name="ps_init", bufs=1, space="PSUM") as ps_init:
            psB = ps_init.tile([T, 81], f32, tag="init", name="psB")
            nc.tensor.matmul(psB[:], tr9bT[:], B9sel[0:T, :],
                             start=True, stop=True, skip_group_check=True)
            Bsb = scratch.tile([T, 81], bf16, name="Bsb")
            nc.vector.tensor_copy(Bsb[:], psB[:])
            psT81 = ps_init.tile([81, 81], f32, tag="init2", name="psT81")
            nc.tensor.matmul(psT81[:], B9sel[0:T, :], Bsb[:],
                             start=True, stop=True, skip_group_check=True)
            E81pre = scratch.tile([81, 81], f32, name="E81pre")
            nc.scalar.activation(E81pre[:], psT81[:], AF.Exp, bias=negln9[:])
            nc.vector.tensor_tensor(E81b[:], E81pre[:], maskA[0:81, 0:81],
                                    ALU.mult)

        # ================= persistent buffers =================
        xT = big.tile([P, NTOK], bf16)
        h_all = big.tile([P, 2 * NTOK], bf16)      # (d, s, b), both s-ordered
        emTb = big.tile([T, NTOK], f32)
        eeTb = big.tile([T, NTOK], bf16)
        ee81 = big.tile([81, CL * P], bf16)        # [81, 4096] (it, c, b)
        c_state = big.tile([P, 2 * BL], bf16)

        # ================= phase B: gather + input projections ============
        with tc.tile_pool(name="ps_tr", bufs=2, space="PSUM") as ps_tr, \
             tc.tile_pool(name="ps_g", bufs=2, space="PSUM") as ps_g:
            for ch in (0, 7, 1, 6, 2, 5, 3, 4):
                xg = stage.tile([P, CTOK // P, E], f32, tag="xg", name="xg")
                nc.gpsimd.dma_gather(
                    xg[:], d_embed,
                    ids_sb[:, ch * (CTOK // BL):(ch + 1) * (CTOK // BL)],
                    num_idxs=CTOK, num_idxs_reg=CTOK, elem_size=E)
                pst = ps_tr.tile([P, CTOK], f32, tag="pst", name="pst")
                for j in range(CTOK // P):
                    nc.tensor.matmul(pst[:, j * P:(j + 1) * P], xg[:, j, :],
                                     I128f[:], is_transpose=True,
                                     skip_group_check=True)
                xTc = xT[:, ch * CTOK:(ch + 1) * CTOK]
                if ch % 2 == 0:
                    nc.scalar.copy(xTc, pst[:])
                else:
                    nc.vector.tensor_copy(xTc, pst[:])

            # ================= recurrence (all-sigmoid) =================
            nc.vector.memset(c_state[:], 0.0)
            c_r = c_state.rearrange("p (d b) -> p d b", d=2)
            for t in range(S):
                sF, sB = t, S - 1 - t
                first = (t == 0)
                G = ps_g.tile([P, 8 * BL], f32, tag="G", name="G")
                nc.tensor.matmul(G[:], I128b[:], bias128[:],
                                 start=True, stop=False, skip_group_check=True)
                for dg in range(8):
                    s = sF if dg < 4 else sB
                    nc.tensor.matmul(
                        G[:, dg * BL:(dg + 1) * BL],
                        wih[:, dg * H:(dg + 1) * H],
                        xT[:, s * BL:(s + 1) * BL],
                        start=False, stop=first, skip_group_check=True)
                if not first:
                    hpF = h_all[:, (t - 1) * BL:t * BL]
                    hpB = h_all[:, NTOK + (sB + 1) * BL:NTOK + (sB + 2) * BL]
                    for dg in range(8):
                        hp = hpF if dg < 4 else hpB
                        nc.tensor.matmul(
                            G[:, dg * BL:(dg + 1) * BL],
                            whh[:, dg * H:(dg + 1) * H],
                            hp, start=False, stop=True, skip_group_check=True)
                # all-tanh cell: T_x = tanh(a_x/2) for f,i,o; T_g = tanh(a_g)
                # cs = 2c state: cs_t = (T_f+1)*cs/2 + (T_i+1)*T_g
                # h'' = 2h = (T_o+1)*tanh(cs/2)
                SA = rec.tile([P, 8 * BL], bf16, tag="SA", name="SA")
                SAr = SA.rearrange("p (d g b) -> p d g b", d=2, b=BL)
                Gr = G.rearrange("p (d g b) -> p d g b", d=2, b=BL)
                nc.scalar.activation(SAr[:, :, 0:3, :], Gr[:, :, 0:3, :],
                                     AF.Tanh)         # f,i,g: on the chain
                nc.scalar.activation(SAr[:, :, 3, :], Gr[:, :, 3, :],
                                     AF.Tanh)         # o: hides in DVE window
                v1 = rec.tile([P, 2 * BL], bf16, tag="v1", name="v1")
                v2 = rec.tile([P, 2 * BL], bf16, tag="v2", name="v2")
                nc.vector.scalar_tensor_tensor(
                    v1.rearrange("p (d b) -> p d b", d=2),
                    SAr[:, :, 1, :], 1.0, SAr[:, :, 2, :],
                    ALU.add, ALU.mult)                # (T_i+1)*T_g = 2i*g
                nc.vector.scalar_tensor_tensor(
                    v2.rearrange("p (d b) -> p d b", d=2),
                    SAr[:, :, 0, :], 1.0, c_r,
                    ALU.add, ALU.mult)                # (T_f+1)*cs = 4f*c
                nc.vector.scalar_tensor_tensor(
                    c_state[:], v2[:], 0.5, v1[:], ALU.mult, ALU.add)
                SC = rec.tile([P, 2 * BL], bf16, tag="SC", name="SC")
                nc.scalar.activation(SC[:], c_state[:], AF.Tanh, scale=0.5)
                hdst = AP(h_all.tensor, h_all.offset + t * BL,
                          [[h_all.ap[0][0], P],
                           [NTOK + (sB - t) * BL, 2], [1, BL]])
                nc.vector.scalar_tensor_tensor(
                    hdst, SAr[:, :, 3, :], 1.0,
                    SC.rearrange("p (d b) -> p d b", d=2),
                    ALU.add, ALU.mult)                # (T_o+1)*T_c = 2h

        # ================= tail: emissions, gold score, CRF =================
        with tc.tile_pool(name="ps_em", bufs=2, space="PSUM") as ps_em, \
             tc.tile_pool(name="ps_crf", bufs=2, space="PSUM") as ps_crf, \
             tc.tile_pool(name="ps_misc", bufs=1, space="PSUM") as ps_misc:

            # ---- emissions ----
            for ch in range(NCH):
                pse = ps_em.tile([T, CTOK], f32, tag="pse", name="pse")
                nc.tensor.matmul(pse[:], wem[:, 0:T],
                                 h_all[:, ch * CTOK:(ch + 1) * CTOK],
                                 start=True, stop=False, skip_group_check=True)
                nc.tensor.matmul(pse[:], wem[:, T:2 * T],
                                 h_all[:, NTOK + ch * CTOK:
                                        NTOK + (ch + 1) * CTOK],
                                 start=False, stop=True, skip_group_check=True)
                nc.vector.scalar_tensor_tensor(
                    emTb[:, ch * CTOK:(ch + 1) * CTOK], pse[:],
                    bem_sb[:], bem_sb[:].broadcast_to([T, CTOK]),
                    ALU.add, ALU.bypass)
                nc.scalar.activation(eeTb[:, ch * CTOK:(ch + 1) * CTOK],
                                     pse[:], AF.Exp, bias=bem_sb[:])

            # ---- ee81 broadcast via selector matmuls ----
            for blk in range(NCH):
                psE = ps_em.tile([81, 512], f32, tag="pse", name="psE")
                rhs = AP(eeTb.tensor, eeTb.offset + blk * 4 * BL,
                         [[eeTb.ap[0][0], T], [BL, 4], [CTOK, NCH], [1, BL]])
                nc.tensor.matmul(psE[:], B9sel[0:T, :], rhs,
                                 start=True, stop=True, skip_group_check=True)
                dst = ee81[:, blk * 512:(blk + 1) * 512]
                if blk % 2 == 0:
                    nc.scalar.copy(dst, psE[:])
                else:
                    nc.vector.tensor_copy(dst, psE[:])

            # ---- gold score ----
            emBps = ps_misc.tile([P, 32 * T], f32, tag="misc", name="emBps")
            for ch in range(32):
                nc.tensor.matmul(emBps[:, ch * T:(ch + 1) * T],
                                 emTb[:, ch * P:(ch + 1) * P],
                                 I9f[:], is_transpose=True,
                                 skip_group_check=True)
            ohE = scratch.tile([P, 32 * T], f32, name="ohE")
            nc.vector.tensor_tensor(
                ohE.rearrange("p (c t) -> p c t", t=T),
                TB.unsqueeze(2).broadcast_to([P, 32, T]),
                iota9.unsqueeze(1).broadcast_to([P, 32, T]),
                ALU.is_equal)
            sacc1 = scratch.tile([P, 1], f32, name="sacc1")
            trash1 = scratch.tile([P, 32 * T], f32, name="trash1")
            nc.vector.scalar_tensor_tensor(trash1[:], emBps[:], 1.0, ohE[:],
                                           ALU.mult, ALU.mult,
                                           accum_out=sacc1[:])

            oh81 = scratch.tile([P, 32 * 81], f32, name="oh81")
            nc.vector.tensor_tensor(
                oh81.rearrange("p (c t) -> p c t", t=81),
                pi32.unsqueeze(2).broadcast_to([P, 32, 81]),
                iota81.unsqueeze(1).broadcast_to([P, 32, 81]),
                ALU.is_equal)
            trbps = ps_misc.tile([P, 81], f32, tag="miscB", name="trbps")
            nc.tensor.matmul(trbps[:], ones1[:], trrow[:], start=True,
                             stop=True, skip_group_check=True)
            sacc2 = scratch.tile([P, 1], f32, name="sacc2")
            trash2 = scratch.tile([P, 32 * 81], f32, name="trash2")
            nc.vector.scalar_tensor_tensor(
                trash2.rearrange("p (c t) -> p c t", t=81),
                trbps.unsqueeze(1).broadcast_to([P, 32, 81]), 1.0,
                oh81.rearrange("p (c t) -> p c t", t=81),
                ALU.mult, ALU.mult, accum_out=sacc2[:])
            spart = scratch.tile([P, 1], f32, name="spart")
            nc.vector.tensor_tensor(spart[:], sacc1[:], sacc2[:], ALU.add)
            psS = ps_misc.tile([BL, 1], f32, tag="misc", name="psS")
            nc.tensor.matmul(psS[:], SEL16[:], spart[:],
                             start=True, stop=True, skip_group_check=True)

            oh9s = scratch.tile([BL, T], f32, name="oh9s")
            nc.vector.tensor_tensor(
                oh9s[:], tags_sb[:, 0:1].broadcast_to([BL, T]),
                iota9[0:BL, :], ALU.is_equal)
            oh9e = scratch.tile([BL, T], f32, name="oh9e")
            nc.vector.tensor_tensor(
                oh9e[:], tags_sb[:, S - 1:S].broadcast_to([BL, T]),
                iota9[0:BL, :], ALU.is_equal)
            se1 = scratch.tile([BL, 1], f32, name="se1")
            se2 = scratch.tile([BL, 1], f32, name="se2")
            tr3 = scratch.tile([BL, T], f32, name="tr3")
            tr4 = scratch.tile([BL, T], f32, name="tr4")
            psst = ps_misc.tile([BL, T], f32, tag="miscB", name="psst")
            nc.tensor.matmul(psst[:], ones1[:, 0:BL], st_sb[:],
                             start=True, stop=True, skip_group_check=True)
            nc.vector.scalar_tensor_tensor(tr3[:], psst[:], 1.0, oh9s[:],
                                           ALU.mult, ALU.mult, accum_out=se1[:])
            psen = ps_misc.tile([BL, T], f32, tag="miscB", name="psen")
            nc.tensor.matmul(psen[:], ones1[:, 0:BL], en_sb[:],
                             start=True, stop=True, skip_group_check=True)
            nc.vector.scalar_tensor_tensor(tr4[:], psen[:], 1.0, oh9e[:],
                                           ALU.mult, ALU.mult, accum_out=se2[:])
            segold = scratch.tile([BL, 1], f32, name="segold")
            nc.vector.tensor_tensor(segold[:], se1[:], se2[:], ALU.add)
            gold = scratch.tile([BL, 1], f32, name="gold")
            nc.vector.scalar_tensor_tensor(gold[:], psS[:], 0.0, segold[:],
                                           ALU.add, ALU.add)

            # ---- CRF forward: exp-space chunked scan (bf16) ----
            gcur = rec.tile([81, P], bf16, tag="G81", name="G81")
            nc.vector.tensor_copy(gcur[:], g0diag[0:81, :].broadcast_to([81, P]))
            HP = P // 2
            gh = [gcur[:, 0:HP], gcur[:, HP:P]]
            for it in range(CL):
                gnew = rec.tile([81, P], bf16, tag="G81", name="G81n")
                ghn = [gnew[:, 0:HP], gnew[:, HP:P]]
                for hf in range(2):
                    lo = hf * HP
                    psG = ps_crf.tile([81, HP], f32, tag="psG", name="psG")
                    if it == 0 and hf == 0:
                        nc.vector.tensor_copy(gnew[:, 0:BL], gcur[:, 0:BL])
                        nc.tensor.matmul(psG[:, BL:HP], E81b[:],
                                         gcur[:, BL:HP], start=True,
                                         stop=True, skip_group_check=True)
                        nc.vector.tensor_tensor(
                            gnew[:, BL:HP], psG[:, BL:HP],
                            ee81[:, it * P + BL:it * P + HP], ALU.mult)
                    else:
                        nc.tensor.matmul(psG[:], E81b[:], gh[hf],
                                         start=True, stop=True,
                                         skip_group_check=True)
                        nc.vector.tensor_tensor(
                            ghn[hf], psG[:],
                            ee81[:, it * P + lo:it * P + lo + HP], ALU.mult)
                gh = ghn
                gcur = gnew

            # ---- alpha0 and chunk combine ----
            pse0 = ps_misc.tile([BL, T], f32, tag="miscC", name="pse0")
            nc.tensor.matmul(pse0[:], emTb[:, 0:BL], I9f[:],
                             is_transpose=True, skip_group_check=True)
            ee0 = scratch.tile([BL, T], f32, name="ee0")
            nc.scalar.activation(ee0[:], pse0[:], AF.Exp)
            expst = scratch.tile([1, T], f32, name="expst")
            nc.scalar.activation(expst[:], st_sb[:], AF.Exp)
            psa = ps_misc.tile([BL, T], f32, tag="miscC", name="psa")
            nc.tensor.matmul(psa[:], ones1[:, 0:BL], expst[:], start=True,
                             stop=True, skip_group_check=True)
            alpha = rec.tile([BL, T], f32, tag="alpha", name="alpha0")
            nc.vector.tensor_tensor(alpha[:], psa[:], ee0[:], ALU.mult)

            gf = scratch.tile([81, P], f32, name="gf")
            nc.vector.tensor_copy(gf[:], gcur[:])
            for c in range(NCH):
                psXc = ps_crf.tile([BL, 81], f32, tag="psG", name="psXc")
                nc.tensor.matmul(psXc[:], gf[:, c * BL:(c + 1) * BL],
                                 I81f[:], is_transpose=True,
                                 skip_group_check=True)
                ctmp = scratch.tile([BL, 81], f32, tag="ctmp", name="ctmp")
                nc.vector.scalar_tensor_tensor(
                    ctmp.rearrange("p (i j) -> p i j", j=T),
                    psXc.rearrange("p (i j) -> p i j", j=T), 1.0,
                    alpha.unsqueeze(2).broadcast_to([BL, T, T]),
                    ALU.mult, ALU.mult)
                anew = rec.tile([BL, T], f32, tag="alpha", name="alphan")
                nc.vector.reduce_sum(anew[:],
                                     ctmp.rearrange("p (i j) -> p j i", j=T),
                                     axis=mybir.AxisListType.X)
                alpha = anew

            expen = scratch.tile([1, T], f32, name="expen")
            nc.scalar.activation(expen[:], en_sb[:], AF.Exp)
            psn = ps_misc.tile([BL, T], f32, tag="miscC", name="psn")
            nc.tensor.matmul(psn[:], ones1[:, 0:BL], expen[:], start=True,
                             stop=True, skip_group_check=True)
            az = scratch.tile([BL, T], f32, name="az")
            nc.vector.scalar_tensor_tensor(az[:], psn[:], 1.0, alpha[:],
                                           ALU.mult, ALU.mult)
            zz = scratch.tile([BL, 1], f32, name="zz")
            nc.vector.reduce_sum(zz[:], az[:], axis=mybir.AxisListType.X)
            logz = scratch.tile([BL, 1], f32, name="logz")
            nc.scalar.activation(logz[:], zz[:], AF.Ln)
            ploss = scratch.tile([BL, 1], f32, name="ploss")
            nc.vector.scalar_tensor_tensor(ploss[:], logz[:],
                                           float(S - 1) * K0LOG, gold[:],
                                           ALU.add, ALU.subtract)
            prow = scratch.tile([1, BL], f32, name="prow")
            nc.sync.dma_start(prow[:], ploss[:])
            lsum = scratch.tile([1, 1], f32, name="lsum")
            nc.vector.reduce_sum(lsum[:], prow[:], axis=mybir.AxisListType.X)
            nc.sync.dma_start(d_out, lsum[:])

    nc.compile()
    return nc


def _host_prep(inputs):
    ids = np.asarray(inputs["input_ids"]).astype(np.int64)
    tags = np.asarray(inputs["tags"]).astype(np.int32)
    embed = np.ascontiguousarray(np.asarray(inputs["embed"], dtype=np.float32))

    # all-tanh cell with h''=2h carried: sigmoid gates (f,i,o) use
    # tanh(a/2) -> w_ih,b x0.5 and w_hh x0.25 (extra x0.5 compensates h'');
    # g-gate uses tanh(a) -> w_ih,b x1, w_hh x0.5; emissions w_em x0.5.
    wihT = np.zeros((E, 8 * H), np.float32)
    whhT = np.zeros((H, 8 * H), np.float32)
    biasf = np.zeros((H, 8), np.float32)
    for d, (wi, wh, bi, bh) in enumerate([
            (inputs["w_ih_f"], inputs["w_hh_f"],
             inputs["b_ih_f"], inputs["b_hh_f"]),
            (inputs["w_ih_b"], inputs["w_hh_b"],
             inputs["b_ih_b"], inputs["b_hh_b"])]):
        wi = np.asarray(wi, np.float32)
        wh = np.asarray(wh, np.float32)
        bsum = np.asarray(bi, np.float32) + np.asarray(bh, np.float32)
        for gi, g in enumerate(GORD):
            gs = 1.0 if gi == 2 else 0.5  # g-gate (idx 2) full scale
            dg = d * 4 + gi
            wihT[:, dg * H:(dg + 1) * H] = wi[g * H:(g + 1) * H].T * gs
            whhT[:, dg * H:(dg + 1) * H] = wh[g * H:(g + 1) * H].T * (0.5 * gs)
            biasf[:, dg] = bsum[g * H:(g + 1) * H] * gs
    wemT = np.ascontiguousarray(np.asarray(inputs["w_em"], np.float32).T) * 0.5
    bem = np.asarray(inputs["b_em"], np.float32).reshape(T, 1)
    sten = np.ascontiguousarray(np.stack([
        np.asarray(inputs["start_trans"], np.float32),
        np.asarray(inputs["end_trans"], np.float32)]))
    trans = np.ascontiguousarray(np.asarray(inputs["trans"], np.float32))

    # structural constants
    pidx = np.arange(128)
    cf32 = np.zeros((128, 97), np.float32)
    q = np.arange(81)
    cf32[:81, 0:81] = (q[:, None] // 9 == q[None, :] // 9)     # maskA
    cf32[:, 81:97] = (pidx[:, None] % 16 == np.arange(16)[None, :])  # SEL16
    cbf32 = np.zeros((128, 163), np.float32)
    cbf32[:9, 0:81] = (q[None, :] % 9 == np.arange(9)[:, None])  # B9sel
    cbf32[:81, 81:162] = np.eye(81)                              # I81
    cbf32[:81, 162] = (q % 9 == q // 9)                          # g0diag

    in_maps = []
    for c in range(NCORES):
        sl = slice(c * BL, (c + 1) * BL)
        in_maps.append({
            "ids16": np.ascontiguousarray(ids[sl].astype(np.int16)),
            "tags": np.ascontiguousarray(tags[sl]),
            "embed": embed,
            "wihT": wihT, "whhT": whhT, "biasf": biasf,
            "wemT": wemT, "bem": bem, "sten": sten, "trans": trans,
            "cf32": cf32, "cbf32": cbf32,
        })
    return in_maps


def kernel(**inputs):
    in_maps = _host_prep(inputs)
    if "nc" not in _NC_CACHE:
        _NC_CACHE["nc"] = _build_program()
    nc = _NC_CACHE["nc"]
    from concourse.bass_utils import run_bass_kernel_spmd
    res = run_bass_kernel_spmd(nc, in_maps, core_ids=list(range(NCORES)))
    _NC_CACHE["exec_time_ns"] = res.exec_time_ns
    total = sum(float(r["out"][0, 0]) for r in res.results)
    return np.array(total / B, dtype=np.float32)
